# revision 6
# baseline (speedup 1.0000x reference)
"""MoE layer (B=4,S=2048,D=1024,F=2048,E=8,topK=2, softmax over token axis)
for 8 Trainium2 NeuronCores.

Strategy: paired expert parallelism with an F-split, bf16 operands.
Experts are paired heavy+light; the two cores of a pair each take one
F-half (w1[:, :F/2] / w2[:F/2, :] vs the other half) of BOTH experts of
the pair and process the SAME token sets (the pair's routed tokens).
The host sums the two partial outputs. This balances per-core work at
(max_heavy + max_light)/... tokens:
  per-core PE work = (sA + sB) tokens x 128 rows with sA = global max
  count, sB = max light-member count -- ~2102 effective tokens vs 2182
  for plain expert parallelism.

Per segment (A = heavy expert of the pair, B = light):
  mm1: hT[f, tok] = relu(sum_d w1h[d,f].T @ xT[d, tok] + b1h[f])   (f in half)
  mm2: yT[dcol, tok] = (sum_{f in half} w2h[f, dcol].T @ hT[f, tok]) * wgt[tok]
Host: out[idx_e] = (yT_half0 + yT_half1).T
"""
import os
import sys

for _p in ("/opt/trn_rl_repo", "/root/.axon_site/_ro/trn_rl_repo"):
    if os.path.isdir(_p) and _p not in sys.path:
        sys.path.append(_p)

import numpy as np
import ml_dtypes
import concourse.bass as bass
import concourse.mybir as mybir
from concourse.tile import TileContext
from concourse.bass_utils import run_bass_kernel_spmd

B, S, D, F, E, K = 4, 2048, 1024, 2048, 8, 2
N = B * S
P = 128
FH = F // 2         # F-half per core
SB = 1024           # token superblock (2 PSUM chunks of 512)
DT = mybir.dt.bfloat16
NPDT = ml_dtypes.bfloat16
N_WARM = 8

_cache = {}


def _split_sync_waits(nc, max_waits=1):
    """The walrus build in this env rejects instructions carrying more than
    ~1 sync wait. Hoist extra waits onto same-engine NOPs placed immediately
    before the offending instruction."""
    ctr = 0
    for f in nc.m.functions:
        for blk in f.blocks:
            new_list = []
            changed = False
            for inst in blk.instructions:
                si = inst.sync_info
                ow = list(si.on_wait) if si and si.on_wait else []
                if len(ow) > max_waits:
                    extra, keep = ow[:-max_waits], ow[-max_waits:]
                    for i in range(0, len(extra), max_waits):
                        ctr += 1
                        nop = mybir.InstNoOp(
                            name=f"I-waitsplit-{ctr}",
                            engine=inst.engine,
                            sync_info=mybir.SyncInfo(
                                on_wait=list(extra[i:i + max_waits]), on_update=[]
                            ),
                        )
                        new_list.append(nop)
                    si.on_wait = keep
                    inst.sync_info = si
                    changed = True
                new_list.append(inst)
            if changed:
                blk.instructions = new_list


def _chunks(n):
    out = []
    off = 0
    while n - off > 512:
        out.append((off, 512))
        off += 512
    out.append((off, n - off))
    return out


def _superblocks(c, first=None):
    """Split c tokens into superblocks; a smaller first block (for segment A)
    lets the PE reach steady state on less up-front DMA."""
    out = []
    off = 0
    if first and c - first >= SB:
        out.append((0, first))
        off = first
    while c - off > SB:
        out.append((off, SB))
        off += SB
    out.append((off, c - off))
    return out


def _build(sA, sB):
    """Per-core program: segment A (sA tokens, weight slot a) then segment B
    (sB tokens, slot b). Each slot is one expert's F-half."""
    nc = bass.Bass("TRN2", target_bir_lowering=False, debug=False, num_devices=E)

    NDT = D // P   # 8 d-tiles
    NFH = FH // P  # 8 f-tiles per half
    Relu = mybir.ActivationFunctionType.Relu

    segs = []
    for name, s in (("a", sA), ("b", sB)):
        spad = -(-s // P) * P
        segs.append({
            "name": name, "s": s, "spad": spad,
            "x": nc.dram_tensor(f"x{name}", [D, spad], DT, kind="ExternalInput"),
            "w1": nc.dram_tensor(f"w1{name}", [D, FH], DT, kind="ExternalInput"),
            "w2": nc.dram_tensor(f"w2{name}", [FH, D], DT, kind="ExternalInput"),
            "b1": nc.dram_tensor(f"b1{name}", [P, NFH], mybir.dt.float32,
                                 kind="ExternalInput"),
            "wgt": nc.dram_tensor(f"wgt{name}", [P, spad], mybir.dt.float32,
                                  kind="ExternalInput"),
            "y": nc.dram_tensor(f"y{name}", [D, spad], mybir.dt.float32,
                                kind="ExternalOutput"),
        })

    with TileContext(nc) as tc:
        with tc.tile_pool(name="wpool", bufs=1) as wpool, \
             tc.tile_pool(name="xpool", bufs=1) as xpool, \
             tc.tile_pool(name="hpool", bufs=2) as hpool, \
             tc.tile_pool(name="ypool", bufs=4) as ypool, \
             tc.tile_pool(name="ps1", bufs=4, space="PSUM") as ps1pool, \
             tc.tile_pool(name="ps2", bufs=4, space="PSUM") as ps2pool:

            # warm-up: PE busy from the moment the engine preambles finish
            # (no DMA dependency) so the HAM clock gate is released and the
            # PE isn't idle while the first x/w1 tiles stream in.
            warm = wpool.tile([P, 512], DT, tag="warm")
            nc.vector.memzero(warm[:, :].bitcast(mybir.dt.float32))
            ps_w = ps1pool.tile([P, 512], mybir.dt.float32, tag="ps1")
            for _ in range(N_WARM):
                nc.tensor.matmul(ps_w[:, :], lhsT=warm[:, :P], rhs=warm[:, :],
                                 start=True, stop=True)

            # ---- DMA issue order (in-order HWDGE queue on sync engine) ----
            # seg A: sb0 x d-tiles interleaved with w1a's first f-quarter so
            # the f0/f1 chains unblock after ~1.5MB; then the rest of w1a in
            # quarters, b1/wgt, remaining x, w2a; then all of seg B.
            WQ = FH // 4
            for seg in segs:
                s, spad = seg["s"], seg["spad"]
                sbs = _superblocks(s, first=(512 if seg["name"] == "a" else None))
                seg["sbs"] = sbs
                xt = xpool.tile([P, NDT * s], DT, tag=f"x_{seg['name']}")
                seg["x_all"] = xt
                sb0_off, sb0_len = sbs[0]
                seg["w1_sb"] = {}
                for d in range(NDT):
                    for (co, cl) in _chunks(sb0_len):
                        nc.sync.dma_start(
                            out=xt[:, d * s + co: d * s + co + cl],
                            in_=seg["x"][d * P:(d + 1) * P, co:co + cl],
                        )
                    w = wpool.tile([P, FH], DT, tag=f"w1{seg['name']}_{d}")
                    nc.sync.dma_start(out=w[:, :WQ],
                                      in_=seg["w1"][d * P:(d + 1) * P, :WQ])
                    seg["w1_sb"][d] = w
                for q in range(1, 4):
                    for d in range(NDT):
                        nc.sync.dma_start(
                            out=seg["w1_sb"][d][:, q * WQ:(q + 1) * WQ],
                            in_=seg["w1"][d * P:(d + 1) * P, q * WQ:(q + 1) * WQ])

                b1_sb = wpool.tile([P, NFH], mybir.dt.float32,
                                   tag=f"b1{seg['name']}")
                nc.sync.dma_start(out=b1_sb[:, :], in_=seg["b1"][:, :])
                seg["b1_sb"] = b1_sb
                wgt_sb = wpool.tile([P, spad], mybir.dt.float32,
                                    tag=f"wgt{seg['name']}")
                nc.sync.dma_start(out=wgt_sb[:, :], in_=seg["wgt"][:, :])
                seg["wgt_sb"] = wgt_sb
                # rest of x (superblocks 1+)
                for si, (off, ln) in enumerate(sbs):
                    if si == 0:
                        continue
                    for d in range(NDT):
                        nc.sync.dma_start(
                            out=xt[:, d * s + off: d * s + off + ln],
                            in_=seg["x"][d * P:(d + 1) * P, off: off + ln],
                        )
                seg["w2_sb"] = {}
                for f in range(NFH):
                    w = wpool.tile([P, D], DT, tag=f"w2{seg['name']}_{f}")
                    nc.sync.dma_start(out=w[:, :],
                                      in_=seg["w2"][f * P:(f + 1) * P, :])
                    seg["w2_sb"][f] = w

            # ---- compute ----
            for seg in segs:
                s = seg["s"]
                xt = seg["x_all"]
                for si, (off, ln) in enumerate(seg["sbs"]):
                    chs = _chunks(ln)
                    hT = hpool.tile([P, NFH * SB], DT, tag="hT")
                    # mm1
                    for f in range(NFH):
                        for (co, cl) in chs:
                            ps = ps1pool.tile([P, 512], mybir.dt.float32,
                                              tag="ps1")
                            for d in range(NDT):
                                nc.tensor.matmul(
                                    ps[:, :cl],
                                    lhsT=seg["w1_sb"][d][:, f * P:(f + 1) * P],
                                    rhs=xt[:, d * s + off + co:
                                           d * s + off + co + cl],
                                    start=(d == 0),
                                    stop=(d == NDT - 1),
                                )
                            nc.scalar.activation(
                                hT[:, f * SB + co: f * SB + co + cl],
                                ps[:, :cl], Relu,
                                bias=seg["b1_sb"][:, f:f + 1],
                            )
                    # mm2
                    for dt in range(NDT):
                        for (co, cl) in chs:
                            ps = ps2pool.tile([P, 512], mybir.dt.float32,
                                              tag="ps2")
                            for f in range(NFH):
                                nc.tensor.matmul(
                                    ps[:, :cl],
                                    lhsT=seg["w2_sb"][f][:, dt * P:(dt + 1) * P],
                                    rhs=hT[:, f * SB + co: f * SB + co + cl],
                                    start=(f == 0),
                                    stop=(f == NFH - 1),
                                )
                            y_sb = ypool.tile([P, 512], mybir.dt.float32,
                                              tag="y")
                            nc.vector.tensor_mul(
                                y_sb[:, :cl], ps[:, :cl],
                                seg["wgt_sb"][:, off + co: off + co + cl],
                            )
                            nc.scalar.dma_start(
                                out=seg["y"][dt * P:(dt + 1) * P,
                                             off + co: off + co + cl],
                                in_=y_sb[:, :cl],
                            )
    _split_sync_waits(nc)
    return nc


def _routing(x_flat, gate_w):
    """logits = x @ gate_w; top-2; softmax over token axis (bit-exact via
    jax-CPU einsum when available)."""
    try:
        import jax
        import jax.numpy as jnp
        cpu = jax.devices("cpu")[0]
        with jax.default_device(cpu):
            logits = np.asarray(
                jnp.einsum(
                    "bsd,de->bse",
                    jnp.asarray(x_flat.reshape(B, S, D)),
                    jnp.asarray(gate_w),
                )
            ).reshape(N, E)
    except Exception:
        logits = (x_flat.astype(np.float64) @ gate_w.astype(np.float64)).astype(
            np.float32
        )

    ar = np.arange(N)
    sel1 = logits.argmax(1)
    v1 = logits[ar, sel1]
    l2 = logits.copy()
    l2[ar, sel1] = -np.inf
    sel2 = l2.argmax(1)
    v2 = logits[ar, sel2]

    v = np.stack([v1, v2], 1).reshape(B, S, K)
    m = v.max(axis=1, keepdims=True)
    ev = np.exp(v - m)
    sm = (ev / ev.sum(axis=1, keepdims=True)).reshape(N, K).astype(np.float32)
    return sel1, sel2, sm[:, 0], sm[:, 1]


def _dispatch(inputs):
    """Host routing + per-core input prep.

    Returns (nc, in_maps, pairs, idx, wgt) where pairs[k] = (heavy_e, light_e)
    handled by cores 2k (F-half 0) and 2k+1 (F-half 1)."""
    x = np.ascontiguousarray(np.asarray(inputs["x"], dtype=np.float32))
    gate_w = np.ascontiguousarray(np.asarray(inputs["gate_w"], dtype=np.float32))
    w1 = np.asarray(inputs["w1"], dtype=np.float32)
    b1 = np.asarray(inputs["b1"], dtype=np.float32)
    w2 = np.asarray(inputs["w2"], dtype=np.float32)

    x_flat = x.reshape(N, D)
    sel1, sel2, sm1, sm2 = _routing(x_flat, gate_w)

    idx = []
    wgt = []
    for e in range(E):
        m1 = sel1 == e
        m2 = sel2 == e
        idx_e = np.nonzero(m1 | m2)[0]
        wgt_e = np.where(m1[idx_e], sm1[idx_e], sm2[idx_e]).astype(np.float32)
        idx.append(idx_e)
        wgt.append(wgt_e)

    order = sorted(range(E), key=lambda e: -len(idx[e]))
    pairs = [(order[k], order[E - 1 - k]) for k in range(E // 2)]
    sA = max(len(idx[h]) for h, _ in pairs)
    sB = max(len(idx[l]) for _, l in pairs)

    if (sA, sB) not in _cache:
        _cache[(sA, sB)] = _build(sA, sB)
    nc = _cache[(sA, sB)]

    def _xT(e, s):
        spad = -(-s // P) * P
        ce = len(idx[e])
        xt = np.zeros((D, spad), dtype=NPDT)
        xt[:, :ce] = x_flat[idx[e]].T.astype(NPDT)
        wg = np.zeros(spad, dtype=np.float32)
        wg[:ce] = wgt[e]
        return xt, np.ascontiguousarray(np.broadcast_to(wg[None, :], (P, spad)))

    in_maps = [None] * E
    for k, (he, le) in enumerate(pairs):
        xa, wga = _xT(he, sA)
        xb, wgb = _xT(le, sB)
        for h in range(2):
            sl = slice(h * FH, (h + 1) * FH)
            in_maps[2 * k + h] = {
                "xa": xa, "wgta": wga,
                "xb": xb, "wgtb": wgb,
                "w1a": np.ascontiguousarray(w1[he][:, sl].astype(NPDT)),
                "w2a": np.ascontiguousarray(w2[he][sl, :].astype(NPDT)),
                "b1a": np.ascontiguousarray(b1[he][sl].reshape(FH // P, P).T),
                "w1b": np.ascontiguousarray(w1[le][:, sl].astype(NPDT)),
                "w2b": np.ascontiguousarray(w2[le][sl, :].astype(NPDT)),
                "b1b": np.ascontiguousarray(b1[le][sl].reshape(FH // P, P).T),
            }
    return nc, in_maps, pairs, idx, wgt


def kernel(x, gate_w, w1, b1, w2, b2):
    inputs = {"x": x, "gate_w": gate_w, "w1": w1, "b1": b1, "w2": w2}
    nc, in_maps, pairs, idx, wgt = _dispatch(inputs)
    b2 = np.asarray(b2, dtype=np.float32)

    res = run_bass_kernel_spmd(nc, in_maps, list(range(E)))

    out = np.zeros((N, D), dtype=np.float32)
    for k, (he, le) in enumerate(pairs):
        for e, yname in ((he, "ya"), (le, "yb")):
            ce = len(idx[e])
            ysum = (res.results[2 * k][yname][:, :ce] +
                    res.results[2 * k + 1][yname][:, :ce])
            out[idx[e]] += ysum.T
            if b2[e].any():
                out[idx[e]] += wgt[e][:, None] * b2[e][None, :]
    return out.reshape(B, S, D)


if __name__ == "__main__":
    rng = np.random.default_rng(0)
    inputs = {
        "x": rng.standard_normal((B, S, D)).astype(np.float32),
        "gate_w": (rng.standard_normal((D, E)) * 0.02).astype(np.float32),
        "w1": (rng.standard_normal((E, D, F)) * 0.02).astype(np.float32),
        "b1": np.zeros((E, F), np.float32),
        "w2": (rng.standard_normal((E, F, D)) * 0.02).astype(np.float32),
        "b2": np.zeros((E, D), np.float32),
    }
    out = kernel(**inputs)
    print("out", out.shape, out.dtype, np.abs(out).max())


# revision 10
# speedup vs baseline: 1.0970x; 1.0970x over previous
"""MoE layer (B=4,S=2048,D=1024,F=2048,E=8,topK=2, softmax over token axis)
for 8 Trainium2 NeuronCores.

Strategy: paired expert parallelism with an F-split, bf16 operands.
Experts are paired heavy+light; the two cores of a pair each take one
F-half (w1[:, :F/2] / w2[:F/2, :] vs the other half) of BOTH experts of
the pair and process the SAME token sets (the pair's routed tokens).
The host sums the two partial outputs. This balances per-core work at
(max_heavy + max_light)/... tokens:
  per-core PE work = (sA + sB) tokens x 128 rows with sA = global max
  count, sB = max light-member count -- ~2102 effective tokens vs 2182
  for plain expert parallelism.

Per segment (A = heavy expert of the pair, B = light):
  mm1: hT[f, tok] = relu(sum_d w1h[d,f].T @ xT[d, tok] + b1h[f])   (f in half)
  mm2: yT[dcol, tok] = (sum_{f in half} w2h[f, dcol].T @ hT[f, tok]) * wgt[tok]
Host: out[idx_e] = (yT_half0 + yT_half1).T
"""
import os
import sys

for _p in ("/opt/trn_rl_repo", "/root/.axon_site/_ro/trn_rl_repo"):
    if os.path.isdir(_p) and _p not in sys.path:
        sys.path.append(_p)

import numpy as np
import ml_dtypes
import concourse.bass as bass
import concourse.mybir as mybir
from concourse.tile import TileContext
from concourse.bass_utils import run_bass_kernel_spmd

B, S, D, F, E, K = 4, 2048, 1024, 2048, 8, 2
N = B * S
P = 128
FH = F // 2         # F-half per core
SB = 1024           # token superblock (2 PSUM chunks of 512)
DT = mybir.dt.bfloat16
NPDT = ml_dtypes.bfloat16
N_WARM = 8

_cache = {}


def _split_sync_waits(nc, max_waits=1):
    """The walrus build in this env rejects instructions carrying more than
    ~1 sync wait. Hoist extra waits onto same-engine NOPs placed immediately
    before the offending instruction."""
    ctr = 0
    for f in nc.m.functions:
        for blk in f.blocks:
            new_list = []
            changed = False
            for inst in blk.instructions:
                si = inst.sync_info
                ow = list(si.on_wait) if si and si.on_wait else []
                if len(ow) > max_waits:
                    extra, keep = ow[:-max_waits], ow[-max_waits:]
                    for i in range(0, len(extra), max_waits):
                        ctr += 1
                        nop = mybir.InstNoOp(
                            name=f"I-waitsplit-{ctr}",
                            engine=inst.engine,
                            sync_info=mybir.SyncInfo(
                                on_wait=list(extra[i:i + max_waits]), on_update=[]
                            ),
                        )
                        new_list.append(nop)
                    si.on_wait = keep
                    inst.sync_info = si
                    changed = True
                new_list.append(inst)
            if changed:
                blk.instructions = new_list


def _chunks(n):
    out = []
    off = 0
    while n - off > 512:
        out.append((off, 512))
        off += 512
    out.append((off, n - off))
    return out


def _superblocks(c, first=None, last=None):
    """Split c tokens into superblocks. A smaller first block (segment A)
    lets the PE reach steady state on less up-front DMA; a small last block
    (segment B) keeps the final store trail off the critical path."""
    tail = 0
    if last and c > last + SB:
        tail = last
        c -= last
    out = []
    off = 0
    if first and c - first >= SB:
        out.append((0, first))
        off = first
    while c - off > SB:
        out.append((off, SB))
        off += SB
    out.append((off, c - off))
    if tail:
        out.append((c, tail))
    return out


def _build(sA, sB):
    """Per-core program: segment A (sA tokens, weight slot a) then segment B
    (sB tokens, slot b). Each slot is one expert's F-half."""
    nc = bass.Bass("TRN2", target_bir_lowering=False, debug=False, num_devices=E)

    NDT = D // P   # 8 d-tiles
    NFH = FH // P  # 8 f-tiles per half
    Relu = mybir.ActivationFunctionType.Relu

    segs = []
    for name, s in (("a", sA), ("b", sB)):
        spad = -(-s // P) * P
        segs.append({
            "name": name, "s": s, "spad": spad,
            "x": nc.dram_tensor(f"x{name}", [D, spad], DT, kind="ExternalInput"),
            "w1": nc.dram_tensor(f"w1{name}", [D, FH], DT, kind="ExternalInput"),
            "w2": nc.dram_tensor(f"w2{name}", [FH, D], DT, kind="ExternalInput"),
            "b1": nc.dram_tensor(f"b1{name}", [P, NFH], mybir.dt.float32,
                                 kind="ExternalInput"),
            "wgt": nc.dram_tensor(f"wgt{name}", [P, spad], mybir.dt.float32,
                                  kind="ExternalInput"),
            "y": nc.dram_tensor(f"y{name}", [D, spad], mybir.dt.float32,
                                kind="ExternalOutput"),
        })

    with TileContext(nc) as tc:
        with tc.tile_pool(name="wpool", bufs=1) as wpool, \
             tc.tile_pool(name="xpool", bufs=1) as xpool, \
             tc.tile_pool(name="hpool", bufs=2) as hpool, \
             tc.tile_pool(name="ypool", bufs=4) as ypool, \
             tc.tile_pool(name="ps1", bufs=4, space="PSUM") as ps1pool, \
             tc.tile_pool(name="ps2", bufs=4, space="PSUM") as ps2pool:

            # warm-up: PE busy from the moment the engine preambles finish
            # (no DMA dependency) so the HAM clock gate is released and the
            # PE isn't idle while the first x/w1 tiles stream in.
            warm = wpool.tile([P, 512], DT, tag="warm")
            nc.vector.memzero(warm[:, :].bitcast(mybir.dt.float32))
            ps_w = ps1pool.tile([P, 512], mybir.dt.float32, tag="ps1")
            for _ in range(N_WARM):
                nc.tensor.matmul(ps_w[:, :], lhsT=warm[:, :P], rhs=warm[:, :],
                                 start=True, stop=True)

            # ---- DMA issue order (in-order HWDGE queue on sync engine) ----
            # Each dma_start costs ~0.6us of queue time regardless of size,
            # so batch into few multi-dim (partition, d, token) transfers:
            # seg A: x sb0 -> w1a in quarters -> b1 -> w2a halves -> wgt ->
            # x rest; then all of seg B (consumed much later).
            WQ = FH // 4
            for seg in segs:
                s, spad = seg["s"], seg["spad"]
                first = seg["name"] == "a"
                last = seg["name"] == "b"
                sbs = _superblocks(s, first=(512 if first else None),
                                   last=(128 if last else None))
                seg["sbs"] = sbs
                xt = xpool.tile([P, NDT * s], DT, tag=f"x_{seg['name']}")
                seg["x_all"] = xt
                xt3 = xt[:, :].rearrange("p (d t) -> p d t", d=NDT)
                xs3 = seg["x"][:, :].rearrange("(d p) t -> p d t", p=P)
                w1t = wpool.tile([P, NDT * FH], DT, tag=f"w1{seg['name']}")
                seg["w1t"] = w1t
                w13 = w1t[:, :].rearrange("p (d f) -> p d f", d=NDT)
                w1s = seg["w1"][:, :].rearrange("(d p) f -> p d f", p=P)
                w2t = wpool.tile([P, NFH * D], DT, tag=f"w2{seg['name']}")
                seg["w2t"] = w2t
                w23 = w2t[:, :].rearrange("p (f c) -> p f c", f=NFH)
                w2s = seg["w2"][:, :].rearrange("(f p) c -> p f c", p=P)

                sb0_len = sbs[0][1]
                nc.sync.dma_start(out=xt3[:, :, :sb0_len],
                                  in_=xs3[:, :, :sb0_len])
                for q in range(4):
                    nc.sync.dma_start(
                        out=w13[:, :, q * WQ:(q + 1) * WQ],
                        in_=w1s[:, :, q * WQ:(q + 1) * WQ])
                b1_sb = wpool.tile([P, NFH], mybir.dt.float32,
                                   tag=f"b1{seg['name']}")
                nc.sync.dma_start(out=b1_sb[:, :], in_=seg["b1"][:, :])
                seg["b1_sb"] = b1_sb
                for h in range(2):
                    nc.sync.dma_start(
                        out=w23[:, h * (NFH // 2):(h + 1) * (NFH // 2), :],
                        in_=w2s[:, h * (NFH // 2):(h + 1) * (NFH // 2), :])
                wgt_sb = wpool.tile([P, spad], mybir.dt.float32,
                                    tag=f"wgt{seg['name']}")
                nc.sync.dma_start(out=wgt_sb[:, :], in_=seg["wgt"][:, :])
                seg["wgt_sb"] = wgt_sb
                # rest of x, one DMA per superblock
                for (off, ln) in sbs[1:]:
                    nc.sync.dma_start(out=xt3[:, :, off:off + ln],
                                      in_=xs3[:, :, off:off + ln])

            # ---- compute ----
            for seg in segs:
                s = seg["s"]
                xt = seg["x_all"]
                w1t, w2t = seg["w1t"], seg["w2t"]
                n_sbs = len(seg["sbs"])
                for si, (off, ln) in enumerate(seg["sbs"]):
                    tiny = seg["name"] == "b" and si == n_sbs - 1
                    chs = _chunks(ln)
                    hT = hpool.tile([P, NFH * SB], DT, tag="hT")
                    # mm1
                    for f in range(NFH):
                        for (co, cl) in chs:
                            ps = ps1pool.tile([P, 512], mybir.dt.float32,
                                              tag="ps1")
                            for d in range(NDT):
                                nc.tensor.matmul(
                                    ps[:, :cl],
                                    lhsT=w1t[:, d * FH + f * P: d * FH + (f + 1) * P],
                                    rhs=xt[:, d * s + off + co:
                                           d * s + off + co + cl],
                                    start=(d == 0),
                                    stop=(d == NDT - 1),
                                )
                            nc.scalar.activation(
                                hT[:, f * SB + co: f * SB + co + cl],
                                ps[:, :cl], Relu,
                                bias=seg["b1_sb"][:, f:f + 1],
                            )
                    # mm2: one batched store per (dtile, superblock); the
                    # final (tiny) superblock gets a single 3D store so the
                    # kernel tail is one small DMA.
                    y_last = None
                    if tiny:
                        y_last = wpool.tile([P, NDT * 128], mybir.dt.float32,
                                            tag="ylast")
                    for dt in range(NDT):
                        y_sb = None if tiny else ypool.tile(
                            [P, SB], mybir.dt.float32, tag="y")
                        for (co, cl) in chs:
                            ps = ps2pool.tile([P, 512], mybir.dt.float32,
                                              tag="ps2")
                            for f in range(NFH):
                                nc.tensor.matmul(
                                    ps[:, :cl],
                                    lhsT=w2t[:, f * D + dt * P: f * D + (dt + 1) * P],
                                    rhs=hT[:, f * SB + co: f * SB + co + cl],
                                    start=(f == 0),
                                    stop=(f == NFH - 1),
                                )
                            dst = (y_last[:, dt * ln: dt * ln + cl] if tiny
                                   else y_sb[:, co:co + cl])
                            nc.vector.tensor_mul(
                                dst, ps[:, :cl],
                                seg["wgt_sb"][:, off + co: off + co + cl],
                            )
                        if not tiny:
                            nc.scalar.dma_start(
                                out=seg["y"][dt * P:(dt + 1) * P, off: off + ln],
                                in_=y_sb[:, :ln],
                            )
                    if tiny:
                        y3 = seg["y"][:, :].rearrange(
                            "(d p) t -> p d t", p=P)[:, :, off:off + ln]
                        nc.scalar.dma_start(
                            out=y3,
                            in_=y_last[:, :].rearrange(
                                "p (d t) -> p d t", d=NDT)[:, :, :ln],
                        )
    _split_sync_waits(nc)
    return nc


def _routing(x_flat, gate_w):
    """logits = x @ gate_w; top-2; softmax over token axis (bit-exact via
    jax-CPU einsum when available)."""
    try:
        import jax
        import jax.numpy as jnp
        cpu = jax.devices("cpu")[0]
        with jax.default_device(cpu):
            logits = np.asarray(
                jnp.einsum(
                    "bsd,de->bse",
                    jnp.asarray(x_flat.reshape(B, S, D)),
                    jnp.asarray(gate_w),
                )
            ).reshape(N, E)
    except Exception:
        logits = (x_flat.astype(np.float64) @ gate_w.astype(np.float64)).astype(
            np.float32
        )

    ar = np.arange(N)
    sel1 = logits.argmax(1)
    v1 = logits[ar, sel1]
    l2 = logits.copy()
    l2[ar, sel1] = -np.inf
    sel2 = l2.argmax(1)
    v2 = logits[ar, sel2]

    v = np.stack([v1, v2], 1).reshape(B, S, K)
    m = v.max(axis=1, keepdims=True)
    ev = np.exp(v - m)
    sm = (ev / ev.sum(axis=1, keepdims=True)).reshape(N, K).astype(np.float32)
    return sel1, sel2, sm[:, 0], sm[:, 1]


def _dispatch(inputs):
    """Host routing + per-core input prep.

    Returns (nc, in_maps, pairs, idx, wgt) where pairs[k] = (heavy_e, light_e)
    handled by cores 2k (F-half 0) and 2k+1 (F-half 1)."""
    x = np.ascontiguousarray(np.asarray(inputs["x"], dtype=np.float32))
    gate_w = np.ascontiguousarray(np.asarray(inputs["gate_w"], dtype=np.float32))
    w1 = np.asarray(inputs["w1"], dtype=np.float32)
    b1 = np.asarray(inputs["b1"], dtype=np.float32)
    w2 = np.asarray(inputs["w2"], dtype=np.float32)

    x_flat = x.reshape(N, D)
    sel1, sel2, sm1, sm2 = _routing(x_flat, gate_w)

    idx = []
    wgt = []
    for e in range(E):
        m1 = sel1 == e
        m2 = sel2 == e
        idx_e = np.nonzero(m1 | m2)[0]
        wgt_e = np.where(m1[idx_e], sm1[idx_e], sm2[idx_e]).astype(np.float32)
        idx.append(idx_e)
        wgt.append(wgt_e)

    order = sorted(range(E), key=lambda e: -len(idx[e]))
    pairs = [(order[k], order[E - 1 - k]) for k in range(E // 2)]
    sA = max(len(idx[h]) for h, _ in pairs)
    sB = max(len(idx[l]) for _, l in pairs)

    if (sA, sB) not in _cache:
        _cache[(sA, sB)] = _build(sA, sB)
    nc = _cache[(sA, sB)]

    def _xT(e, s):
        spad = -(-s // P) * P
        ce = len(idx[e])
        xt = np.zeros((D, spad), dtype=NPDT)
        xt[:, :ce] = x_flat[idx[e]].T.astype(NPDT)
        wg = np.zeros(spad, dtype=np.float32)
        wg[:ce] = wgt[e]
        return xt, np.ascontiguousarray(np.broadcast_to(wg[None, :], (P, spad)))

    in_maps = [None] * E
    for k, (he, le) in enumerate(pairs):
        xa, wga = _xT(he, sA)
        xb, wgb = _xT(le, sB)
        for h in range(2):
            sl = slice(h * FH, (h + 1) * FH)
            in_maps[2 * k + h] = {
                "xa": xa, "wgta": wga,
                "xb": xb, "wgtb": wgb,
                "w1a": np.ascontiguousarray(w1[he][:, sl].astype(NPDT)),
                "w2a": np.ascontiguousarray(w2[he][sl, :].astype(NPDT)),
                "b1a": np.ascontiguousarray(b1[he][sl].reshape(FH // P, P).T),
                "w1b": np.ascontiguousarray(w1[le][:, sl].astype(NPDT)),
                "w2b": np.ascontiguousarray(w2[le][sl, :].astype(NPDT)),
                "b1b": np.ascontiguousarray(b1[le][sl].reshape(FH // P, P).T),
            }
    return nc, in_maps, pairs, idx, wgt


def kernel(x, gate_w, w1, b1, w2, b2):
    inputs = {"x": x, "gate_w": gate_w, "w1": w1, "b1": b1, "w2": w2}
    nc, in_maps, pairs, idx, wgt = _dispatch(inputs)
    b2 = np.asarray(b2, dtype=np.float32)

    res = run_bass_kernel_spmd(nc, in_maps, list(range(E)))

    out = np.zeros((N, D), dtype=np.float32)
    for k, (he, le) in enumerate(pairs):
        for e, yname in ((he, "ya"), (le, "yb")):
            ce = len(idx[e])
            ysum = (res.results[2 * k][yname][:, :ce] +
                    res.results[2 * k + 1][yname][:, :ce])
            out[idx[e]] += ysum.T
            if b2[e].any():
                out[idx[e]] += wgt[e][:, None] * b2[e][None, :]
    return out.reshape(B, S, D)


if __name__ == "__main__":
    rng = np.random.default_rng(0)
    inputs = {
        "x": rng.standard_normal((B, S, D)).astype(np.float32),
        "gate_w": (rng.standard_normal((D, E)) * 0.02).astype(np.float32),
        "w1": (rng.standard_normal((E, D, F)) * 0.02).astype(np.float32),
        "b1": np.zeros((E, F), np.float32),
        "w2": (rng.standard_normal((E, F, D)) * 0.02).astype(np.float32),
        "b2": np.zeros((E, D), np.float32),
    }
    out = kernel(**inputs)
    print("out", out.shape, out.dtype, np.abs(out).max())
